# revision 1
# baseline (speedup 1.0000x reference)
"""LinearCapsPro forward on 8 TRN2 NeuronCores.

Math: out[b,c] = sqrt(u^T sigma u), u = W_c x_b, sigma = (W_c W_c^T + eps I)^-1.
Host-side fold: G_c = W_c W_c^T + eps I = L_c L_c^T  =>  u^T G^-1 u = ||L_c^-1 u||^2.
With W'_c = L_c^-1 W_c the device kernel is just v = x @ W'^T, then
out[b,c] = sqrt(sum_d v[b, c*16+d]^2) - one big matmul + square + group-sum + sqrt.

Sharding: data-parallel over batch (512 rows/core), W' replicated; no collectives.

Schedule (per core):
  - x^T [2048,512] bf16 loaded as 16 k-pieces on the ACT HW-DGE ring.
  - W'^T [2048,1600] bf16 loaded as 32 (k, col-half) pieces on the SP ring,
    first-half columns first so stripe-0 compute can start ~2us in.
  - Compute loops stripe(4 x 400 cd-cols) -> k(16) -> m(4 x 128 batch rows):
    4 PSUM banks live per stripe (double-buffered across stripes = 8 banks).
  - Epilogue per (stripe, m): ACT square psum->sbuf, DVE group-sum(16) into a
    per-m [128,100] result tile; final ACT sqrt + one output DMA per m.
"""

import sys

import numpy as np

try:
    import concourse  # noqa: F401
except ImportError:  # fresh grading dir: concourse lives in the RL repo
    sys.path.insert(0, "/opt/trn_rl_repo")

B, F, C, D = 4096, 2048, 100, 16
N_CORES = 8
BL = B // N_CORES  # 512 batch rows per core
CD = C * D  # 1600
EPS = 1e-4
KT = F // 128  # 16 contraction tiles
MT = BL // 128  # 4 batch tiles per core
NS = 400  # cd-stripe width (uniform; 4 stripes; 25 capsules each)
ST = CD // NS  # 4 stripes

_cached_nc = None


def build_bass(repeat=1, last_m_outer=True, sqrt_inline=False, m_outer_from=None, ep_bufs=4, fine_head=False, no_epilogue=False):
    """repeat>1 builds a NEFF with the compute body repeated (same output) —
    used only for launch-overhead-immune slope timing, never for grading."""
    import concourse.bacc as bacc
    import concourse.mybir as mybir
    import concourse.tile as tile

    fp16 = mybir.dt.float16
    f32 = mybir.dt.float32
    nc = bacc.Bacc("TRN2", target_bir_lowering=False, debug=False, num_devices=N_CORES)
    xT = nc.dram_tensor("xT", [F, BL], fp16, kind="ExternalInput")
    wT = nc.dram_tensor("wT", [F, CD], fp16, kind="ExternalInput")
    out = nc.dram_tensor("out", [BL, C], f32, kind="ExternalOutput")

    with tile.TileContext(nc) as tc:
        with (
            tc.tile_pool(name="xp", bufs=1) as xp,
            tc.tile_pool(name="wp", bufs=1) as wp,
            tc.tile_pool(name="ps", bufs=2, space="PSUM") as psp,
            tc.tile_pool(name="ep", bufs=ep_bufs) as ep,
            tc.tile_pool(name="rp", bufs=1) as rp,
        ):
            # x on the ACT ring (parallel to w's SP ring), split into 4
            # k-group pieces so the first matmuls only wait ~1.5us
            xsb = xp.tile([128, KT, BL], fp16)
            xk_edges = [0, 1, 3, 5, 7, 9, 11, 13, 16] if fine_head else [
                g * 2 for g in range(9)
            ]
            for a, b in zip(xk_edges[:-1], xk_edges[1:]):
                nc.scalar.dma_start(
                    xsb[:, a:b, :],
                    xT[a * 128 : b * 128, :].rearrange("(k p) m -> p k m", p=128),
                )
            # w: stripe-major SBUF layout [128, stripe, k, 400]. Stripe 0 is
            # split in two k-halves (PE can start on k0-7 early); stripes 1-3
            # are one 1.6MB DMA each, prefetching behind compute on the FIFO ring
            wsb = wp.tile([128, ST, KT, NS], fp16)
            wk_edges = [0, 1, 4, 8, 12, 16] if fine_head else [0, 4, 8, 12, 16]
            for a, b in zip(wk_edges[:-1], wk_edges[1:]):
                nc.sync.dma_start(
                    wsb[:, 0, a:b, :],
                    wT[a * 128 : b * 128, 0:NS].rearrange("(k p) n -> p k n", p=128),
                )
            for s in range(1, ST):
                nc.sync.dma_start(
                    wsb[:, s, :, :],
                    wT[:, s * NS : (s + 1) * NS].rearrange("(k p) n -> p k n", p=128),
                )
            for r in range(repeat):
                res = [
                    rp.tile([128, C], f32, tag=f"res{m}", name=f"res_r{r}_m{m}")
                    for m in range(MT)
                ]
                for s in range(ST):
                    pss = [
                        psp.tile([128, NS], f32, tag=f"ps{m}", name=f"ps_s{s}_m{m}")
                        for m in range(MT)
                    ]
                    # k-outer on early stripes (matches x/w DMA arrival order);
                    # m-outer on the last stripe so m0-m2's epilogues overlap the
                    # remaining matmuls instead of serializing after them
                    mof = (ST - 1) if m_outer_from is None else m_outer_from
                    if s < mof or not last_m_outer:
                        order = [(k, m) for k in range(KT) for m in range(MT)]
                    else:
                        order = [(k, m) for m in range(MT) for k in range(KT)]
                    for k, m in order:
                        nc.tensor.matmul(
                            pss[m][:],
                            xsb[:, k, m * 128 : (m + 1) * 128],  # lhsT [K, M]
                            wsb[:, s, k, :],  # rhs [K, N]
                            start=(k == 0),
                            stop=(k == KT - 1),
                        )
                    ncaps = NS // D  # 25
                    if no_epilogue:
                        continue
                    for m in range(MT):
                        sq = ep.tile([128, NS], f32, tag="sq")
                        nc.scalar.square(sq[:], pss[m][:])
                        nc.vector.reduce_sum(
                            res[m][:, s * ncaps : (s + 1) * ncaps],
                            sq[:].rearrange("p (c d) -> p c d", d=D),
                            axis=mybir.AxisListType.X,
                        )
                        if sqrt_inline and s == ST - 1:
                            nc.scalar.sqrt(res[m][:], res[m][:])
                            nc.scalar.dma_start(
                                out[m * 128 : (m + 1) * 128, :], res[m][:]
                            )
                if no_epilogue:
                    for m in range(MT):
                        tmp = ep.tile([128, C], f32, tag="sq", name=f"tmp{m}")
                        nc.vector.tensor_copy(tmp[:], pss[m][:, :C])
                        nc.scalar.dma_start(out[m * 128 : (m + 1) * 128, :], tmp[:])
                elif not sqrt_inline:
                    for m in range(MT):
                        nc.scalar.sqrt(res[m][:], res[m][:])
                        nc.scalar.dma_start(
                            out[m * 128 : (m + 1) * 128, :], res[m][:]
                        )
    nc.compile()
    return nc


def prep_inputs(x: np.ndarray, weight: np.ndarray):
    """Host-side fold + shard. Returns in_maps for the 8 cores."""
    W64 = weight.astype(np.float64)  # [C, D, F]
    G = np.einsum("cdf,cef->cde", W64, W64)
    G[:, np.arange(D), np.arange(D)] += EPS
    L = np.linalg.cholesky(G)
    Wp = np.linalg.solve(L, W64)  # L^-1 W : [C, D, F]
    wT = np.ascontiguousarray(Wp.reshape(CD, F).T.astype(np.float16))  # [F, CD]
    xT = np.ascontiguousarray(x.T.astype(np.float16))  # [F, B]
    return [
        {"xT": np.ascontiguousarray(xT[:, i * BL : (i + 1) * BL]), "wT": wT}
        for i in range(N_CORES)
    ]


def kernel(x: np.ndarray, weight: np.ndarray) -> np.ndarray:
    global _cached_nc
    x = np.asarray(x)
    weight = np.asarray(weight)
    assert x.shape == (B, F) and weight.shape == (C, D, F), (x.shape, weight.shape)
    in_maps = prep_inputs(x, weight)
    if _cached_nc is None:
        _cached_nc = build_bass()
    from concourse.bass_utils import run_bass_kernel_spmd

    res = run_bass_kernel_spmd(_cached_nc, in_maps, core_ids=list(range(N_CORES)))
    return np.concatenate(
        [res.results[i]["out"] for i in range(N_CORES)], axis=0
    ).astype(np.float32)

